# revision 1
# baseline (speedup 1.0000x reference)
import numpy as np
import concourse.bass as bass
import concourse.bacc as bacc
import concourse.mybir as mybir
from concourse import tile
from concourse.bass_utils import run_bass_kernel_spmd

NX = 2048
NY = 2048
NCORES = 8
R = NX // NCORES
SLAB = R + 2
YP = NY + 2

TAU = 0.6
INV_TAU = 1.0 / TAU
FCOEF = 1.0 - INV_TAU
W1P = INV_TAU * (1.0 / 9.0)
W5P = INV_TAU * (1.0 / 36.0)
W0P = INV_TAU * (4.0 / 9.0)

EX = [0, 1, 0, -1, 0, 1, -1, -1, 1]
EY = [0, 0, 1, 0, -1, 1, 1, -1, -1]
OPP = [0, 3, 4, 1, 2, 7, 8, 5, 6]

W = 512
NCHUNK = NY // W
F = W + 2
FP32 = mybir.dt.float32
BF16 = mybir.dt.bfloat16
U8 = mybir.dt.uint8

TILE_BASES = [0, 130]
FX_FS0 = 126
FX_NFS = 6
FX_OUT0 = 127
FX_NOUT = 4
FX_SEG = 8
FX_W = NY // FX_SEG
FX_F = FX_W + 2
FX_PO = FX_SEG * FX_NOUT
FX_PF = FX_SEG * FX_NFS
SHM_COLS = 256 + 3 * FX_PO

AL = mybir.AluOpType


def _v3(ap):
    return ap.rearrange("p (x c) -> p x c", c=1)


def _collide(nc, scr, P, FW, fu12, rhoT, fstar, tagp):
    vec = nc.vector
    rv_f32 = _v3(rhoT[:])

    def t(name, dt=BF16):
        tl = scr.tile([P, FW], dt, tag=f"{tagp}{name}")
        return _v3(tl[:])

    ff = []
    for c in range(9):
        ffc = t(f"in{c}")
        nc.scalar.mul(ffc, fu12[:, :, c:c + 1], FCOEF)
        ff.append(ffc)
    ux = t("inux"); uy = t("inuy"); rvb = t("inrho")
    nc.scalar.copy(ux, fu12[:, :, 9:10])
    nc.scalar.copy(uy, fu12[:, :, 10:11])
    nc.scalar.copy(rvb, rv_f32)
    mkbt = scr.tile([P, FW], BF16, tag=f"{tagp}inmkb")
    nc.scalar.copy(_v3(mkbt[:]), fu12[:, :, 11:12])
    mku = scr.tile([P, FW], U8, tag=f"{tagp}mku")
    nc.scalar.copy(_v3(mku[:]), fu12[:, :, 11:12])

    r1 = t("r1"); r2 = t("r2"); t1 = t("t1"); t2 = t("t2")
    usqr = t("usqr"); Pv = t("P", FP32); sv = t("s"); dv = t("d")
    rsv = t("rs"); rdv = t("rd"); a5 = t("A5"); a6 = t("A6")
    pw9 = t("pw9"); pw36 = t("pw36"); pw49 = t("pw49", FP32)
    g1 = t("G1"); g2 = t("G2"); g5 = t("G5"); g6 = t("G6")
    aa1 = t("AA1"); aa2 = t("AA2"); aa5 = t("AA5"); aa6 = t("AA6")
    rr1 = t("RR1"); rr2 = t("RR2"); rr5 = t("RR5"); rr6 = t("RR6")
    feqv = []
    for i in range(1, 9):
        fq = t(f"feq{i}")
        feqv.append(fq)

    vec.tensor_tensor(r1, rvb, ux, AL.mult)
    vec.tensor_tensor(r2, rvb, uy, AL.mult)
    vec.tensor_tensor(t1, ux, r1, AL.mult)
    vec.tensor_tensor(t2, uy, r2, AL.mult)
    vec.tensor_tensor(usqr, t1, t2, AL.add)
    vec.scalar_tensor_tensor(Pv, usqr, -1.5, rv_f32, AL.mult, AL.add)
    vec.tensor_tensor(sv, ux, uy, AL.add)
    vec.tensor_tensor(dv, ux, uy, AL.subtract)
    vec.tensor_tensor(rsv, r1, r2, AL.add)
    vec.tensor_tensor(rdv, r1, r2, AL.subtract)
    vec.tensor_tensor(a5, sv, rsv, AL.mult)
    vec.tensor_tensor(a6, dv, rdv, AL.mult)
    nc.scalar.mul(pw9, Pv, W1P)
    nc.scalar.mul(pw36, Pv, W5P)
    nc.scalar.mul(pw49, Pv, W0P)
    vec.tensor_scalar_mul(aa1, t1, 4.5 * W1P)
    vec.tensor_scalar_mul(aa2, t2, 4.5 * W1P)
    vec.tensor_scalar_mul(aa5, a5, 4.5 * W5P)
    vec.tensor_scalar_mul(aa6, a6, 4.5 * W5P)
    vec.tensor_scalar_mul(rr1, r1, 3 * W1P)
    vec.tensor_scalar_mul(rr2, r2, 3 * W1P)
    vec.tensor_scalar_mul(rr5, rsv, 3 * W5P)
    vec.tensor_scalar_mul(rr6, rdv, 3 * W5P)
    vec.tensor_tensor(g1, aa1, pw9, AL.add)
    vec.tensor_tensor(g2, aa2, pw9, AL.add)
    vec.tensor_tensor(g5, aa5, pw36, AL.add)
    vec.tensor_tensor(g6, aa6, pw36, AL.add)
    vec.tensor_tensor(feqv[0], g1, rr1, AL.add)
    vec.tensor_tensor(feqv[2], g1, rr1, AL.subtract)
    vec.tensor_tensor(feqv[1], g2, rr2, AL.add)
    vec.tensor_tensor(feqv[3], g2, rr2, AL.subtract)
    vec.tensor_tensor(feqv[4], g5, rr5, AL.add)
    vec.tensor_tensor(feqv[6], g5, rr5, AL.subtract)
    vec.tensor_tensor(feqv[5], g6, rr6, AL.subtract)
    vec.tensor_tensor(feqv[7], g6, rr6, AL.add)
    fsv = [_v3(fs[:]) for fs in fstar]
    vec.tensor_tensor(fsv[0], ff[0], pw49, AL.add)
    for i in range(1, 9):
        vec.tensor_tensor(fsv[i], ff[i], feqv[i - 1], AL.add)
    return fsv, mku, mkbt


def _lift(nc, scr, P, OW, ov, tagp):
    vec = nc.vector

    def t(name):
        tl = scr.tile([P, OW], FP32, tag=f"{tagp}{name}")
        return _v3(tl[:])

    av = t("feq1"); bv = t("feq2"); cv = t("feq3"); ddv = t("feq4")
    m1 = t("feq5"); m2 = t("feq6")
    t01 = t("feq7"); t23 = t("feq8"); t45 = t("G1"); t67 = t("G2")
    t03 = t("G5"); t47 = t("G6"); t07 = t("pw9"); inv = t("pw36")
    rhoF = t("lrho")
    s_ = [ov[:, :, i:i + 1] for i in range(9)]
    vec.tensor_tensor(av, s_[1], s_[3], AL.subtract)
    vec.tensor_tensor(bv, s_[2], s_[4], AL.subtract)
    vec.tensor_tensor(cv, s_[5], s_[7], AL.subtract)
    vec.tensor_tensor(ddv, s_[8], s_[6], AL.subtract)
    vec.tensor_tensor(m1, av, cv, AL.add)
    vec.tensor_tensor(m1, m1, ddv, AL.add)
    vec.tensor_tensor(m2, bv, cv, AL.add)
    vec.tensor_tensor(m2, m2, ddv, AL.subtract)
    vec.tensor_tensor(t01, s_[0], s_[1], AL.add)
    vec.tensor_tensor(t23, s_[2], s_[3], AL.add)
    vec.tensor_tensor(t45, s_[4], s_[5], AL.add)
    vec.tensor_tensor(t67, s_[6], s_[7], AL.add)
    vec.tensor_tensor(t03, t01, t23, AL.add)
    vec.tensor_tensor(t47, t45, t67, AL.add)
    vec.tensor_tensor(t07, t03, t47, AL.add)
    vec.tensor_tensor(rhoF, t07, s_[8], AL.add)
    nc.scalar.copy(ov[:, :, 9:10], rhoF)
    vec.reciprocal_approx_fast(inv, rhoF)
    vec.tensor_tensor(ov[:, :, 10:11], m1, inv, AL.mult)
    vec.tensor_tensor(ov[:, :, 11:12], m2, inv, AL.mult)


def _build_program():
    nc = bacc.Bacc(None)

    fu_d = nc.declare_dram_parameter("fu", [SLAB, YP, 12], BF16, isOutput=False)
    rho_d = nc.declare_dram_parameter("rho", [SLAB, YP], FP32, isOutput=False)
    shm_d = nc.declare_dram_parameter("shm", [128, SHM_COLS], BF16, isOutput=False)
    out_d = nc.declare_dram_parameter("out", [R, NY, 12], BF16, isOutput=True)

    with tile.TileContext(nc) as tc, tc.tile_pool(name="cst", bufs=1) as cst:
        shm = cst.tile([128, SHM_COLS], BF16)
        nc.sync.dma_start(out=shm[:], in_=shm_d[:, :])
        with (
            tc.tile_pool(name="io", bufs=2) as io,
            tc.tile_pool(name="pln", bufs=1) as pln,
            tc.tile_pool(name="psS", bufs=1, space="PSUM") as psS,
            tc.tile_pool(name="scr", bufs=1) as scr,
        ):
            it = 0
            for tb in TILE_BASES:
                for ch in range(NCHUNK):
                    c0 = ch * W
                    fuT = io.tile([128, F * 12], BF16, tag="fuT")
                    rhoT = io.tile([128, F], FP32, tag="rhoT")
                    outT = io.tile([128, W * 12], BF16, tag="outT")
                    nc.sync.dma_start(out=fuT[:], in_=fu_d[tb:tb + 128, c0:c0 + F, :].rearrange("r y c -> r (y c)"))
                    nc.sync.dma_start(out=rhoT[:], in_=rho_d[tb:tb + 128, c0:c0 + F])
                    fstar = [pln.tile([128, F], BF16, tag=f"fs{i}", name=f"fs{i}")
                             for i in range(9)]

                    fu12 = fuT[:].rearrange("p (x c) -> p x c", c=12)
                    fsv, mku, _mkb = _collide(nc, scr, 128, F, fu12, rhoT, fstar, "m_")

                    ov = outT[:].rearrange("p (x c) -> p x c", c=12)
                    mk = _v3(mku[:])[:, 1:1 + W, :]
                    for i in range(9):
                        exi, eyi = EX[i], EY[i]
                        ysl = slice(1 - eyi, 1 - eyi + W)
                        if exi == 0:
                            src = fsv[i][:, ysl, :]
                        else:
                            sp = psS.tile([128, W], FP32, tag=f"S{i}",
                                          name=f"S{i}")
                            wcol = slice(0, 128) if exi == 1 else slice(128, 256)
                            nc.tensor.matmul(sp[:], shm[:, wcol],
                                             fstar[i][:, ysl])
                            src = _v3(sp[:])
                        nc.scalar.copy(ov[:, :, i:i + 1], src)
                    for i in range(1, 9):
                        nc.vector.copy_predicated(ov[:, :, i:i + 1], mk,
                                                  fsv[OPP[i]][:, 1:1 + W, :])

                    _lift(nc, scr, 128, W, ov, "m_")

                    st_eng = nc.sync
                    st_eng.dma_start(
                        out=out_d[tb:tb + 126, c0:c0 + W, :].rearrange(
                            "r y c -> r (y c)"),
                        in_=outT[1:127, :])
                    it += 1

            PF = FX_PF
            PO = FX_PO
            fxfu = io.tile([PF, FX_F * 12], BF16, tag="fuT")
            fxrho = io.tile([PF, FX_F], FP32, tag="rhoT")
            fxout = io.tile([PO, FX_W * 12], BF16, tag="outT")
            for sg in range(FX_SEG):
                c0 = sg * FX_W
                nc.sync.dma_start(
                    out=fxfu[sg * FX_NFS:(sg + 1) * FX_NFS, :],
                    in_=fu_d[FX_FS0:FX_FS0 + FX_NFS, c0:c0 + FX_F, :].rearrange(
                        "r y c -> r (y c)"))
                nc.sync.dma_start(
                    out=fxrho[sg * FX_NFS:(sg + 1) * FX_NFS, :],
                    in_=rho_d[FX_FS0:FX_FS0 + FX_NFS, c0:c0 + FX_F])
            fxstar = [pln.tile([PF, FX_F], BF16, tag=f"fs{i}", name=f"fxs{i}")
                      for i in range(9)]

            fv12 = fxfu[:].rearrange("p (x c) -> p x c", c=12)
            _, _fxmku, fxmkb = _collide(nc, scr, PF, FX_F, fv12, fxrho, fxstar, "m_")

            ov = fxout[:].rearrange("p (x c) -> p x c", c=12)
            PBASE = {1: 256, 0: 256 + FX_PO, -1: 256 + 2 * FX_PO}
            for i in range(9):
                exi, eyi = EX[i], EY[i]
                ysl = slice(1 - eyi, 1 - eyi + FX_W)
                sp = psS.tile([PO, FX_W], FP32, tag="fxSp", name=f"fxS{i}")
                b = PBASE[exi]
                nc.tensor.matmul(sp[:], shm[0:PF, b:b + PO],
                                 fxstar[i][:, ysl])
                nc.scalar.copy(ov[:, :, i:i + 1], _v3(sp[:]))
            mkps = psS.tile([PO, FX_W], FP32, tag="fxBp", name="fxMk")
            nc.tensor.matmul(mkps[:], shm[0:PF, 256 + FX_PO:256 + 2 * FX_PO],
                             fxmkb[:, 1:1 + FX_W])
            fxmaskP = pln.tile([PO, FX_W], U8, tag="fxmaskP")
            nc.scalar.copy(_v3(fxmaskP[:]), _v3(mkps[:]))
            mk = _v3(fxmaskP[:])
            for i in range(1, 9):
                spb = psS.tile([PO, FX_W], FP32, tag="fxBp", name=f"fxB{i}")
                nc.tensor.matmul(spb[:], shm[0:PF, 256 + FX_PO:256 + 2 * FX_PO],
                                 fxstar[OPP[i]][:, 1:1 + FX_W])
                nc.vector.copy_predicated(ov[:, :, i:i + 1], mk, _v3(spb[:]))

            _lift(nc, scr, PO, FX_W, ov, "m_")

            for sg in range(FX_SEG):
                st_eng = nc.sync
                st_eng.dma_start(
                    out=out_d[FX_OUT0 - 1:FX_OUT0 - 1 + FX_NOUT,
                              sg * FX_W:(sg + 1) * FX_W, :].rearrange(
                        "r y c -> r (y c)"),
                    in_=fxout[sg * FX_NOUT:(sg + 1) * FX_NOUT, :])

    nc.finalize()
    return nc


_NC_CACHE = None


def _get_nc():
    global _NC_CACHE
    if _NC_CACHE is None:
        _NC_CACHE = _build_program()
    return _NC_CACHE


def _shm_np():
    import ml_dtypes
    m = np.zeros((128, SHM_COLS), np.float32)
    for i in range(1, 128):
        m[i - 1, i] = 1.0
    for i in range(0, 127):
        m[i + 1, 128 + i] = 1.0
    for bi, exi in enumerate((1, 0, -1)):
        base = 256 + FX_PO * bi
        for sg in range(FX_SEG):
            for jj in range(FX_NOUT):
                q = sg * FX_NOUT + jj
                k = sg * FX_NFS + jj + 1 - exi
                m[k, base + q] = 1.0
    return m.astype(ml_dtypes.bfloat16)


def _pad_slab(arr, lo, hi):
    rows = np.take(arr, np.arange(lo - 1, hi + 1), axis=0, mode="wrap")
    return np.concatenate([rows[:, -1:], rows, rows[:, :1]], axis=1)


def kernel(f, rho, u, obstacle_mask, _trace=False):
    import ml_dtypes
    f = np.asarray(f, dtype=np.float32)
    rho = np.asarray(rho, dtype=np.float32)
    u = np.asarray(u, dtype=np.float32)
    maskf = np.asarray(obstacle_mask).astype(np.float32)
    fu = np.concatenate([f, u, maskf[..., None]],
                        axis=-1).astype(ml_dtypes.bfloat16)

    shm = _shm_np()
    in_maps = []
    for k in range(NCORES):
        lo, hi = k * R, (k + 1) * R
        in_maps.append({
            "fu": np.ascontiguousarray(_pad_slab(fu, lo, hi)),
            "rho": np.ascontiguousarray(_pad_slab(rho, lo, hi)),
            "shm": shm,
        })

    nc = _get_nc()
    res = run_bass_kernel_spmd(nc, in_maps, list(range(NCORES)),
                               trace=bool(_trace))
    out = np.concatenate([res.results[k]["out"] for k in range(NCORES)],
                     axis=0).astype(np.float32)
    if _trace:
        return out, res
    return out



# revision 6
# speedup vs baseline: 1.6417x; 1.6417x over previous
import numpy as np
import concourse.bass as bass
import concourse.bacc as bacc
import concourse.mybir as mybir
from concourse import tile
from concourse.bass_utils import run_bass_kernel_spmd

NX = 2048
NY = 2048
NCORES = 8
R = NX // NCORES
SLAB = R + 2
YP = NY + 2
TB = [0, 130]
W = 512
NCH = NY // W

TAU = 0.6
INV_TAU = 1.0 / TAU
FCOEF = 1.0 - INV_TAU
W1P = INV_TAU * (1.0 / 9.0)
W5P = INV_TAU * (1.0 / 36.0)
W0P = INV_TAU * (4.0 / 9.0)

EX = [0, 1, 0, -1, 0, 1, -1, -1, 1]
EY = [0, 0, 1, 0, -1, 1, 1, -1, -1]
OPP = [0, 3, 4, 1, 2, 7, 8, 5, 6]

FXR0 = 126
FXNR = 6
FXSEG = 8
FXW = NY // FXSEG
FXF = FXW + 2
FXP = FXSEG * FXNR

FP32 = mybir.dt.float32
BF16 = mybir.dt.bfloat16
U8 = mybir.dt.uint8
AL = mybir.AluOpType

def _pamaoa(ey):
    pa = 1 + ey
    return pa, pa, 1


def _build_program():
    nc = bacc.Bacc(None)

    fin_d = nc.declare_dram_parameter("fin", [12, SLAB, YP], BF16, isOutput=False)
    mk_d = nc.declare_dram_parameter("mk", [3, SLAB, YP], U8, isOutput=False)
    wts_d = nc.declare_dram_parameter("wts", [128, 4 * 128], BF16, isOutput=False)
    pfin_d = nc.declare_dram_parameter("pfin", [12, FXP, FXF], BF16, isOutput=False)
    pmk_d = nc.declare_dram_parameter("pmk", [3, FXP, FXF], U8, isOutput=False)
    pwts_d = nc.declare_dram_parameter("pwts", [FXP, 4 * FXP], BF16, isOutput=False)
    out_d = nc.declare_dram_parameter("out", [12, SLAB, NY], BF16, isOutput=True)

    def tt(eng, o, a, b, op):
        eng.tensor_tensor(o, a, b, op)

    with tile.TileContext(nc) as tc, tc.tile_pool(name="cst", bufs=1) as cst:
        wts = cst.tile([128, 4 * 128], BF16)
        pwts = cst.tile([FXP, 4 * FXP], BF16)
        nc.sync.dma_start(out=wts[:], in_=wts_d[:, :])
        nc.sync.dma_start(out=pwts[:], in_=pwts_d[:, :])
        def wblk(s):
            return wts[:, (s + 1) * 128:(s + 2) * 128]
        def pwblk(s):
            return pwts[:, (s + 1) * FXP:(s + 2) * FXP]

        with (
            tc.tile_pool(name="io", bufs=2) as io,
            tc.tile_pool(name="mki", bufs=1) as mki,
            tc.tile_pool(name="o2", bufs=1) as o2p,
            tc.tile_pool(name="psS", bufs=1, space="PSUM") as psS,
            tc.tile_pool(name="scr", bufs=1) as scr,
        ):
            for tb in TB:
                IN = io.tile([128, 12 * YP], BF16, tag="IN")
                MK = mki.tile([128, 3 * YP], U8, tag="MK")
                OUT2 = o2p.tile([128, 3 * NY], BF16, tag="OUT2")
                nc.sync.dma_start(
                    out=IN[:].rearrange("p (c y) -> p c y", c=12),
                    in_=fin_d[:, tb:tb + 128, :].rearrange("c p y -> p c y"))
                nc.sync.dma_start(
                    out=MK[:].rearrange("p (c y) -> p c y", c=3),
                    in_=mk_d[:, tb:tb + 128, :].rearrange("c p y -> p c y"))

                def F(i, a=0, b=YP):
                    return IN[:, i * YP + a:i * YP + b]
                UX = F(9); UY = F(10); RH = F(11)
                def MKV(s, a, b):
                    return MK[:, (s + 1) * YP + a:(s + 1) * YP + b]

                def S(name, dt=BF16, wdt=YP):
                    return scr.tile([128, wdt], dt, tag=name, name=name)

                r1 = S("r1"); r2 = S("r2"); t1 = S("t1"); t2 = S("t2")
                sv = S("sv"); dv = S("dv"); rs = S("rs"); rd = S("rd")
                a5 = S("a5"); a6 = S("a6"); uq = S("uq")
                V = nc.vector; P = nc.gpsimd

                tt(P, r1[:], RH, UX, AL.mult)
                tt(P, r2[:], RH, UY, AL.mult)
                tt(P, t1[:], UX, r1[:], AL.mult)
                tt(P, t2[:], UY, r2[:], AL.mult)
                tt(V, uq[:], t1[:], t2[:], AL.add)
                V.tensor_scalar_mul(uq[:], uq[:], -1.5)
                tt(V, uq[:], uq[:], RH, AL.add)
                tt(P, sv[:], UX, UY, AL.add)
                tt(P, dv[:], UX, UY, AL.subtract)
                tt(V, rs[:], r1[:], r2[:], AL.add)
                tt(V, rd[:], r1[:], r2[:], AL.subtract)
                tt(P, a5[:], sv[:], rs[:], AL.mult)
                tt(P, a6[:], dv[:], rd[:], AL.mult)
                V.tensor_scalar_mul(t1[:], t1[:], 4.5 * W1P)
                V.tensor_scalar_mul(t2[:], t2[:], 4.5 * W1P)
                V.tensor_scalar_mul(a5[:], a5[:], 4.5 * W5P)
                V.tensor_scalar_mul(a6[:], a6[:], 4.5 * W5P)
                V.tensor_scalar_mul(r1[:], r1[:], 3.0 * W1P)
                V.tensor_scalar_mul(r2[:], r2[:], 3.0 * W1P)
                V.tensor_scalar_mul(rs[:], rs[:], 3.0 * W5P)
                V.tensor_scalar_mul(rd[:], rd[:], 3.0 * W5P)
                V.tensor_scalar_mul(sv[:], uq[:], W1P)
                V.tensor_scalar_mul(dv[:], uq[:], W5P)
                V.tensor_scalar_mul(uq[:], uq[:], W0P)
                tt(P, t1[:], t1[:], sv[:], AL.add)
                tt(P, t2[:], t2[:], sv[:], AL.add)
                tt(P, a5[:], a5[:], dv[:], AL.add)
                tt(P, a6[:], a6[:], dv[:], AL.add)
                tt(V, F(0), F(0), uq[:], AL.add)
                tt(V, F(1), F(1), t1[:], AL.add)
                tt(V, F(1), F(1), r1[:], AL.add)
                tt(V, F(3), F(3), t1[:], AL.add)
                tt(V, F(3), F(3), r1[:], AL.subtract)
                tt(V, F(2), F(2), t2[:], AL.add)
                tt(V, F(2), F(2), r2[:], AL.add)
                tt(P, F(4), F(4), t2[:], AL.add)
                tt(P, F(4), F(4), r2[:], AL.subtract)
                tt(P, F(5), F(5), a5[:], AL.add)
                tt(P, F(5), F(5), rs[:], AL.add)
                tt(P, F(7), F(7), a5[:], AL.add)
                tt(P, F(7), F(7), rs[:], AL.subtract)
                tt(P, F(6), F(6), a6[:], AL.add)
                tt(P, F(6), F(6), rd[:], AL.subtract)
                tt(P, F(8), F(8), a6[:], AL.add)
                tt(P, F(8), F(8), rd[:], AL.add)

                for i in range(1, 9):
                    exi, eyi = EX[i], EY[i]
                    pa, ma, oa = _pamaoa(eyi)
                    bb = S(f"bb{i % 2}", wdt=NY)
                    for c in range(NCH):
                        sp = psS.tile([128, W], FP32, tag=f"sp{c % 4}",
                                      name=f"sp{i}_{c}")
                        nc.tensor.matmul(sp[:], wblk(exi),
                                         F(OPP[i], ma + W * c, ma + W * (c + 1)))
                        nc.scalar.copy(bb[:, W * c:W * (c + 1)], sp[:])
                    V.copy_predicated(F(i, oa, oa + NY),
                                      MKV(exi, pa, pa + NY), bb[:])

                RA = S("r1"); RB = S("r2"); R0 = S("t1")
                MA_ = S("t2"); M0 = S("a5"); MB = S("a6")
                tt(P, RA[:, 0:NY], F(1, 1, 1 + NY), F(5, 0, NY), AL.add)
                tt(P, RA[:, 0:NY], RA[:, 0:NY], F(8, 2, 2 + NY), AL.add)
                tt(P, RB[:, 0:NY], F(3, 1, 1 + NY), F(6, 0, NY), AL.add)
                tt(P, RB[:, 0:NY], RB[:, 0:NY], F(7, 2, 2 + NY), AL.add)
                tt(V, R0[:, 0:NY], F(0, 1, 1 + NY), F(2, 0, NY), AL.add)
                tt(V, R0[:, 0:NY], R0[:, 0:NY], F(4, 2, 2 + NY), AL.add)
                tt(V, MA_[:, 0:NY], F(5, 0, NY), F(8, 2, 2 + NY), AL.subtract)
                tt(P, M0[:, 0:NY], F(2, 0, NY), F(4, 2, 2 + NY), AL.subtract)
                tt(P, MB[:, 0:NY], F(6, 0, NY), F(7, 2, 2 + NY), AL.subtract)

                sm1 = S("sv", wdt=NY); sm2 = S("dv", wdt=NY)
                inv = S("inv", FP32, wdt=NY)
                for c in range(NCH):
                    cs = slice(W * c, W * (c + 1))
                    rp = psS.tile([128, W], FP32, tag="sp0", name=f"rp{c}")
                    nc.tensor.matmul(rp[:], wblk(-1), RA[:, cs], start=True, stop=False)
                    nc.tensor.matmul(rp[:], wblk(0), R0[:, cs], start=False, stop=False)
                    nc.tensor.matmul(rp[:], wblk(1), RB[:, cs], start=False, stop=True)
                    m1p = psS.tile([128, W], FP32, tag="sp1", name=f"m1p{c}")
                    nc.tensor.matmul(m1p[:], wblk(-1), RA[:, cs], start=True, stop=False)
                    nc.tensor.matmul(m1p[:], wts[:, 3 * 128:4 * 128], RB[:, cs],
                                     start=False, stop=True)
                    m2p = psS.tile([128, W], FP32, tag="sp2", name=f"m2p{c}")
                    nc.tensor.matmul(m2p[:], wblk(-1), MA_[:, cs], start=True, stop=False)
                    nc.tensor.matmul(m2p[:], wblk(0), M0[:, cs], start=False, stop=False)
                    nc.tensor.matmul(m2p[:], wblk(1), MB[:, cs], start=False, stop=True)
                    nc.scalar.copy(OUT2[:, cs], rp[:])
                    V.reciprocal_approx_fast(inv[:, cs], rp[:])
                    nc.scalar.copy(sm1[:, cs], m1p[:])
                    nc.scalar.copy(sm2[:, cs], m2p[:])
                invb = S("invb", wdt=NY)
                nc.scalar.copy(invb[:], inv[:])
                tt(V, OUT2[:, NY:2 * NY], sm1[:], invb[:], AL.mult)
                tt(V, OUT2[:, 2 * NY:3 * NY], sm2[:], invb[:], AL.mult)

                nc.sync.dma_start(
                    out=out_d[0:9, tb:tb + 128, :].rearrange("c p y -> p c y"),
                    in_=IN[:].rearrange("p (c y) -> p c y", c=12)[:, 0:9, 1:1 + NY])
                nc.sync.dma_start(
                    out=out_d[9:12, tb + 1:tb + 127, :].rearrange("c p y -> p c y"),
                    in_=OUT2[1:127, :].rearrange("p (c y) -> p c y", c=3))

            pIN = mki.tile([FXP, 12 * FXF], BF16, tag="pIN")
            pMK = mki.tile([FXP, 3 * FXF], U8, tag="pMK")
            nc.sync.dma_start(
                out=pIN[:].rearrange("p (c y) -> p c y", c=12),
                in_=pfin_d[:, :, :].rearrange("c p y -> p c y"))
            nc.sync.dma_start(
                out=pMK[:].rearrange("p (c y) -> p c y", c=3),
                in_=pmk_d[:, :, :].rearrange("c p y -> p c y"))

            def pF(i, a=0, b=FXF):
                return pIN[:, i * FXF + a:i * FXF + b]
            pUX = pF(9); pUY = pF(10); pRH = pF(11)
            def pMKV(s, a, b):
                return pMK[:, (s + 1) * FXF + a:(s + 1) * FXF + b]

            def PS(name, dt=BF16, wdt=FXF):
                return scr.tile([FXP, wdt], dt, tag=f"p_{name}", name=f"p_{name}")

            r1 = PS("r1"); r2 = PS("r2"); t1 = PS("t1"); t2 = PS("t2")
            sv = PS("sv"); dv = PS("dv"); rs = PS("rs"); rd = PS("rd")
            a5 = PS("a5"); a6 = PS("a6"); uq = PS("uq")
            V = nc.vector; P = nc.gpsimd

            tt(P, r1[:], pRH, pUX, AL.mult)
            tt(P, r2[:], pRH, pUY, AL.mult)
            tt(P, t1[:], pUX, r1[:], AL.mult)
            tt(P, t2[:], pUY, r2[:], AL.mult)
            tt(V, uq[:], t1[:], t2[:], AL.add)
            V.tensor_scalar_mul(uq[:], uq[:], -1.5)
            tt(V, uq[:], uq[:], pRH, AL.add)
            tt(P, sv[:], pUX, pUY, AL.add)
            tt(P, dv[:], pUX, pUY, AL.subtract)
            tt(V, rs[:], r1[:], r2[:], AL.add)
            tt(V, rd[:], r1[:], r2[:], AL.subtract)
            tt(P, a5[:], sv[:], rs[:], AL.mult)
            tt(P, a6[:], dv[:], rd[:], AL.mult)
            V.tensor_scalar_mul(t1[:], t1[:], 4.5 * W1P)
            V.tensor_scalar_mul(t2[:], t2[:], 4.5 * W1P)
            V.tensor_scalar_mul(a5[:], a5[:], 4.5 * W5P)
            V.tensor_scalar_mul(a6[:], a6[:], 4.5 * W5P)
            V.tensor_scalar_mul(r1[:], r1[:], 3.0 * W1P)
            V.tensor_scalar_mul(r2[:], r2[:], 3.0 * W1P)
            V.tensor_scalar_mul(rs[:], rs[:], 3.0 * W5P)
            V.tensor_scalar_mul(rd[:], rd[:], 3.0 * W5P)
            V.tensor_scalar_mul(sv[:], uq[:], W1P)
            V.tensor_scalar_mul(dv[:], uq[:], W5P)
            V.tensor_scalar_mul(uq[:], uq[:], W0P)
            tt(P, t1[:], t1[:], sv[:], AL.add)
            tt(P, t2[:], t2[:], sv[:], AL.add)
            tt(P, a5[:], a5[:], dv[:], AL.add)
            tt(P, a6[:], a6[:], dv[:], AL.add)
            tt(V, pF(0), pF(0), uq[:], AL.add)
            tt(V, pF(1), pF(1), t1[:], AL.add)
            tt(V, pF(1), pF(1), r1[:], AL.add)
            tt(V, pF(3), pF(3), t1[:], AL.add)
            tt(V, pF(3), pF(3), r1[:], AL.subtract)
            tt(V, pF(2), pF(2), t2[:], AL.add)
            tt(V, pF(2), pF(2), r2[:], AL.add)
            tt(P, pF(4), pF(4), t2[:], AL.add)
            tt(P, pF(4), pF(4), r2[:], AL.subtract)
            tt(P, pF(5), pF(5), a5[:], AL.add)
            tt(P, pF(5), pF(5), rs[:], AL.add)
            tt(P, pF(7), pF(7), a5[:], AL.add)
            tt(P, pF(7), pF(7), rs[:], AL.subtract)
            tt(P, pF(6), pF(6), a6[:], AL.add)
            tt(P, pF(6), pF(6), rd[:], AL.subtract)
            tt(P, pF(8), pF(8), a6[:], AL.add)
            tt(P, pF(8), pF(8), rd[:], AL.add)

            for i in range(1, 9):
                exi, eyi = EX[i], EY[i]
                pa, ma, oa = _pamaoa(eyi)
                bb = PS(f"bb{i % 2}", wdt=FXW)
                sp = psS.tile([FXP, FXW], FP32, tag="psp0", name=f"psp{i}")
                nc.tensor.matmul(sp[:], pwblk(exi), pF(OPP[i], ma, ma + FXW))
                nc.scalar.copy(bb[:], sp[:])
                V.copy_predicated(pF(i, oa, oa + FXW),
                                  pMKV(exi, pa, pa + FXW), bb[:])

            RA = PS("r1"); RB = PS("r2"); R0 = PS("t1")
            MA_ = PS("t2"); M0 = PS("a5"); MB = PS("a6")
            tt(P, RA[:, 0:FXW], pF(1, 1, 1 + FXW), pF(5, 0, FXW), AL.add)
            tt(P, RA[:, 0:FXW], RA[:, 0:FXW], pF(8, 2, 2 + FXW), AL.add)
            tt(P, RB[:, 0:FXW], pF(3, 1, 1 + FXW), pF(6, 0, FXW), AL.add)
            tt(P, RB[:, 0:FXW], RB[:, 0:FXW], pF(7, 2, 2 + FXW), AL.add)
            tt(V, R0[:, 0:FXW], pF(0, 1, 1 + FXW), pF(2, 0, FXW), AL.add)
            tt(V, R0[:, 0:FXW], R0[:, 0:FXW], pF(4, 2, 2 + FXW), AL.add)
            tt(V, MA_[:, 0:FXW], pF(5, 0, FXW), pF(8, 2, 2 + FXW), AL.subtract)
            tt(P, M0[:, 0:FXW], pF(2, 0, FXW), pF(4, 2, 2 + FXW), AL.subtract)
            tt(P, MB[:, 0:FXW], pF(6, 0, FXW), pF(7, 2, 2 + FXW), AL.subtract)

            rp = psS.tile([FXP, FXW], FP32, tag="psp0", name="prp")
            nc.tensor.matmul(rp[:], pwblk(-1), RA[:, 0:FXW], start=True, stop=False)
            nc.tensor.matmul(rp[:], pwblk(0), R0[:, 0:FXW], start=False, stop=False)
            nc.tensor.matmul(rp[:], pwblk(1), RB[:, 0:FXW], start=False, stop=True)
            m1p = psS.tile([FXP, FXW], FP32, tag="psp1", name="pm1p")
            nc.tensor.matmul(m1p[:], pwblk(-1), RA[:, 0:FXW], start=True, stop=False)
            nc.tensor.matmul(m1p[:], pwts[:, 3 * FXP:4 * FXP], RB[:, 0:FXW],
                             start=False, stop=True)
            m2p = psS.tile([FXP, FXW], FP32, tag="psp2", name="pm2p")
            nc.tensor.matmul(m2p[:], pwblk(-1), MA_[:, 0:FXW], start=True, stop=False)
            nc.tensor.matmul(m2p[:], pwblk(0), M0[:, 0:FXW], start=False, stop=False)
            nc.tensor.matmul(m2p[:], pwblk(1), MB[:, 0:FXW], start=False, stop=True)
            pinv = PS("pinv", FP32, wdt=FXW)
            psm1 = PS("sv", wdt=FXW); psm2 = PS("dv", wdt=FXW)
            nc.scalar.copy(pF(9, 1, 1 + FXW), rp[:])
            V.reciprocal_approx_fast(pinv[:], rp[:])
            nc.scalar.copy(psm1[:], m1p[:])
            nc.scalar.copy(psm2[:], m2p[:])
            pinvb = PS("pinvb", wdt=FXW)
            nc.scalar.copy(pinvb[:], pinv[:])
            tt(V, pF(10, 1, 1 + FXW), psm1[:], pinvb[:], AL.mult)
            tt(V, pF(11, 1, 1 + FXW), psm2[:], pinvb[:], AL.mult)

            for sg in range(FXSEG):
                nc.sync.dma_start(
                    out=out_d[:, 127:131, sg * FXW:(sg + 1) * FXW].rearrange(
                        "c r y -> r c y"),
                    in_=pIN[sg * FXNR + 1:sg * FXNR + 5, :].rearrange(
                        "p (c y) -> p c y", c=12)[:, :, 1:1 + FXW])

    nc.finalize()
    return nc


_NC_CACHE = None


def _get_nc():
    global _NC_CACHE
    if _NC_CACHE is None:
        _NC_CACHE = _build_program()
    return _NC_CACHE


def _wts_np():
    import ml_dtypes
    m = np.zeros((128, 4 * 128), np.float32)
    for s in (-1, 0, 1):
        for q in range(128):
            k = q + s
            if 0 <= k < 128:
                m[k, (s + 1) * 128 + q] = 1.0
    for q in range(128):
        k = q + 1
        if 0 <= k < 128:
            m[k, 3 * 128 + q] = -1.0
    return m.astype(ml_dtypes.bfloat16)


def _pwts_np():
    import ml_dtypes
    m = np.zeros((FXP, 4 * FXP), np.float32)
    for s in (-1, 0, 1):
        for sg in range(FXSEG):
            for j in range(FXNR):
                q = sg * FXNR + j
                jk = j + s
                if 0 <= jk < FXNR:
                    m[sg * FXNR + jk, (s + 1) * FXP + q] = 1.0
    for sg in range(FXSEG):
        for j in range(FXNR):
            q = sg * FXNR + j
            jk = j + 1
            if 0 <= jk < FXNR:
                m[sg * FXNR + jk, 3 * FXP + q] = -1.0
    return m.astype(ml_dtypes.bfloat16)


def _prep_inputs(f, rho, u, obstacle_mask):
    import ml_dtypes
    f = np.asarray(f, dtype=np.float32)
    rho = np.asarray(rho, dtype=np.float32)
    u = np.asarray(u, dtype=np.float32)
    mask = np.asarray(obstacle_mask).astype(np.uint8)

    planes = np.empty((12, NX, NY), np.float32)
    for i in range(9):
        planes[i] = FCOEF * f[..., i]
    planes[9] = u[..., 0]
    planes[10] = u[..., 1]
    planes[11] = rho
    planes_b = planes.astype(ml_dtypes.bfloat16)

    wts = _wts_np()
    pwts = _pwts_np()
    rows_idx = np.arange(-1, R + 1)
    cols_idx = np.arange(-1, NY + 1) % NY
    in_maps = []
    for k in range(NCORES):
        lo = k * R
        ridx = (lo + rows_idx) % NX
        fin = planes_b[:, ridx][:, :, cols_idx]
        mk = np.empty((3, SLAB, YP), np.uint8)
        for si, s in enumerate((-1, 0, 1)):
            mk[si] = mask[(lo + rows_idx + s) % NX][:, cols_idx]
        pfin = np.empty((12, FXP, FXF), ml_dtypes.bfloat16)
        pmk = np.empty((3, FXP, FXF), np.uint8)
        frows = (lo - 1 + FXR0 + np.arange(FXNR)) % NX
        for sg in range(FXSEG):
            ccols = (sg * FXW + np.arange(-1, FXW + 1)) % NY
            seg = planes_b[:, frows][:, :, ccols]
            pfin[:, sg * FXNR:(sg + 1) * FXNR] = seg
            for si, s in enumerate((-1, 0, 1)):
                pmk[si, sg * FXNR:(sg + 1) * FXNR] = \
                    mask[(frows + s) % NX][:, ccols]
        in_maps.append({
            "fin": np.ascontiguousarray(fin),
            "mk": np.ascontiguousarray(mk),
            "wts": wts,
            "pfin": np.ascontiguousarray(pfin),
            "pmk": np.ascontiguousarray(pmk),
            "pwts": pwts,
        })
    return in_maps


def kernel(f, rho, u, obstacle_mask, _trace=False):
    in_maps = _prep_inputs(f, rho, u, obstacle_mask)
    nc = _get_nc()
    res = run_bass_kernel_spmd(nc, in_maps, list(range(NCORES)),
                               trace=bool(_trace))
    full = np.empty((NX, NY, 12), np.float32)
    chan = np.concatenate(
        [np.asarray(res.results[k]["out"])[:, 1:R + 1, :].astype(np.float32)
         for k in range(NCORES)], axis=1)
    for c in range(9):
        full[..., c] = np.roll(chan[c], (EX[c], EY[c]), axis=(0, 1))
    full[..., 9] = chan[9]
    full[..., 10] = chan[10]
    full[..., 11] = chan[11]
    if _trace:
        return full, res
    return full


# revision 8
# speedup vs baseline: 1.6518x; 1.0061x over previous
import numpy as np
import concourse.bass as bass
import concourse.bacc as bacc
import concourse.mybir as mybir
from concourse import tile
from concourse.bass_utils import run_bass_kernel_spmd

NX = 2048
NY = 2048
NCORES = 8
R = NX // NCORES
SLAB = R + 2
YP = NY + 2
TB = [0, 130]
W = 512
NCH = NY // W

TAU = 0.6
INV_TAU = 1.0 / TAU
FCOEF = 1.0 - INV_TAU
W1P = INV_TAU * (1.0 / 9.0)
W5P = INV_TAU * (1.0 / 36.0)
W0P = INV_TAU * (4.0 / 9.0)

EX = [0, 1, 0, -1, 0, 1, -1, -1, 1]
EY = [0, 0, 1, 0, -1, 1, 1, -1, -1]
OPP = [0, 3, 4, 1, 2, 7, 8, 5, 6]

FXR0 = 126
FXNR = 6
FXSEG = 8
FXW = NY // FXSEG
FXF = FXW + 2
FXP = FXSEG * FXNR

FP32 = mybir.dt.float32
BF16 = mybir.dt.bfloat16
U8 = mybir.dt.uint8
AL = mybir.AluOpType

def _pamaoa(ey):
    pa = 1 + ey
    return pa, pa, 1


def _build_program():
    nc = bacc.Bacc(None)

    fin_d = nc.declare_dram_parameter("fin", [12, SLAB, YP], BF16, isOutput=False)
    mk_d = nc.declare_dram_parameter("mk", [3, SLAB, YP], U8, isOutput=False)
    wts_d = nc.declare_dram_parameter("wts", [128, 4 * 128], BF16, isOutput=False)
    pfin_d = nc.declare_dram_parameter("pfin", [12, FXP, FXF], BF16, isOutput=False)
    pmk_d = nc.declare_dram_parameter("pmk", [3, FXP, FXF], U8, isOutput=False)
    pwts_d = nc.declare_dram_parameter("pwts", [FXP, 4 * FXP], BF16, isOutput=False)
    out_d = nc.declare_dram_parameter("out", [12, SLAB, NY], BF16, isOutput=True)

    def tt(eng, o, a, b, op):
        eng.tensor_tensor(o, a, b, op)

    with tile.TileContext(nc) as tc, tc.tile_pool(name="cst", bufs=1) as cst:
        wts = cst.tile([128, 4 * 128], BF16)
        pwts = cst.tile([FXP, 4 * FXP], BF16)
        nc.sync.dma_start(out=wts[:], in_=wts_d[:, :])
        nc.sync.dma_start(out=pwts[:], in_=pwts_d[:, :])
        def wblk(s):
            return wts[:, (s + 1) * 128:(s + 2) * 128]
        def pwblk(s):
            return pwts[:, (s + 1) * FXP:(s + 2) * FXP]

        with (
            tc.tile_pool(name="io", bufs=2) as io,
            tc.tile_pool(name="mki", bufs=1) as mki,
            tc.tile_pool(name="o2", bufs=1) as o2p,
            tc.tile_pool(name="psS", bufs=1, space="PSUM") as psS,
            tc.tile_pool(name="scr", bufs=1) as scr,
        ):
            for tb in TB:
                IN = io.tile([128, 12 * YP], BF16, tag="IN")
                MK = mki.tile([128, 3 * YP], U8, tag="MK")
                OUT2 = o2p.tile([128, 3 * NY], BF16, tag="OUT2")
                nc.sync.dma_start(
                    out=IN[:].rearrange("p (c y) -> p c y", c=12),
                    in_=fin_d[:, tb:tb + 128, :].rearrange("c p y -> p c y"))
                nc.sync.dma_start(
                    out=MK[:].rearrange("p (c y) -> p c y", c=3),
                    in_=mk_d[:, tb:tb + 128, :].rearrange("c p y -> p c y"))

                def F(i, a=0, b=YP):
                    return IN[:, i * YP + a:i * YP + b]
                UX = F(9); UY = F(10); RH = F(11)
                def MKV(s, a, b):
                    return MK[:, (s + 1) * YP + a:(s + 1) * YP + b]

                def S(name, dt=BF16, wdt=YP):
                    return scr.tile([128, wdt], dt, tag=name, name=name)

                r1 = S("r1"); r2 = S("r2"); t1 = S("t1"); t2 = S("t2")
                sv = S("sv"); dv = S("dv"); rs = S("rs"); rd = S("rd")
                a5 = S("a5"); a6 = S("a6"); uq = S("uq")
                V = nc.vector; P = nc.gpsimd

                tt(P, r1[:], RH, UX, AL.mult)
                tt(P, r2[:], RH, UY, AL.mult)
                tt(P, t1[:], UX, r1[:], AL.mult)
                tt(P, t2[:], UY, r2[:], AL.mult)
                tt(V, uq[:], t1[:], t2[:], AL.add)
                V.tensor_scalar_mul(uq[:], uq[:], -1.5)
                tt(V, uq[:], uq[:], RH, AL.add)
                tt(P, sv[:], UX, UY, AL.add)
                tt(P, dv[:], UX, UY, AL.subtract)
                tt(V, rs[:], r1[:], r2[:], AL.add)
                tt(V, rd[:], r1[:], r2[:], AL.subtract)
                tt(P, a5[:], sv[:], rs[:], AL.mult)
                tt(P, a6[:], dv[:], rd[:], AL.mult)
                V.tensor_scalar_mul(t1[:], t1[:], 4.5 * W1P)
                V.tensor_scalar_mul(t2[:], t2[:], 4.5 * W1P)
                V.tensor_scalar_mul(a5[:], a5[:], 4.5 * W5P)
                V.tensor_scalar_mul(a6[:], a6[:], 4.5 * W5P)
                V.tensor_scalar_mul(r1[:], r1[:], 3.0 * W1P)
                V.tensor_scalar_mul(r2[:], r2[:], 3.0 * W1P)
                V.tensor_scalar_mul(rs[:], rs[:], 3.0 * W5P)
                V.tensor_scalar_mul(rd[:], rd[:], 3.0 * W5P)
                V.tensor_scalar_mul(sv[:], uq[:], W1P)
                V.tensor_scalar_mul(dv[:], uq[:], W5P)
                V.tensor_scalar_mul(uq[:], uq[:], W0P)
                tt(P, t1[:], t1[:], sv[:], AL.add)
                tt(P, t2[:], t2[:], sv[:], AL.add)
                tt(P, a5[:], a5[:], dv[:], AL.add)
                tt(P, a6[:], a6[:], dv[:], AL.add)
                tt(V, F(0), F(0), uq[:], AL.add)
                tt(V, F(1), F(1), t1[:], AL.add)
                tt(V, F(1), F(1), r1[:], AL.add)
                tt(V, F(3), F(3), t1[:], AL.add)
                tt(V, F(3), F(3), r1[:], AL.subtract)
                tt(V, F(2), F(2), t2[:], AL.add)
                tt(V, F(2), F(2), r2[:], AL.add)
                tt(P, F(4), F(4), t2[:], AL.add)
                tt(P, F(4), F(4), r2[:], AL.subtract)
                tt(P, F(5), F(5), a5[:], AL.add)
                tt(P, F(5), F(5), rs[:], AL.add)
                tt(P, F(7), F(7), a5[:], AL.add)
                tt(P, F(7), F(7), rs[:], AL.subtract)
                tt(P, F(6), F(6), a6[:], AL.add)
                tt(P, F(6), F(6), rd[:], AL.subtract)
                tt(P, F(8), F(8), a6[:], AL.add)
                tt(P, F(8), F(8), rd[:], AL.add)

                for i, j in ((1, 3), (2, 4), (5, 7), (6, 8)):
                    bbs = {}
                    for d, tag in ((i, "bb0"), (j, "bb1")):
                        exd, eyd = EX[d], EY[d]
                        pa, ma, oa = _pamaoa(eyd)
                        bb = S(tag, wdt=NY)
                        for c in range(NCH):
                            sp = psS.tile([128, W], FP32, tag=f"sp{c % 4}",
                                          name=f"sp{d}_{c}")
                            nc.tensor.matmul(sp[:], wblk(exd),
                                             F(OPP[d], ma + W * c, ma + W * (c + 1)))
                            nc.scalar.copy(bb[:, W * c:W * (c + 1)], sp[:])
                        bbs[d] = bb
                    for d in (i, j):
                        exd, eyd = EX[d], EY[d]
                        pa, ma, oa = _pamaoa(eyd)
                        V.copy_predicated(F(d, oa, oa + NY),
                                          MKV(exd, pa, pa + NY), bbs[d][:])

                RA = S("r1"); RB = S("r2"); R0 = S("t1")
                MA_ = S("t2"); M0 = S("a5"); MB = S("a6")
                tt(P, RA[:, 0:NY], F(1, 1, 1 + NY), F(5, 0, NY), AL.add)
                tt(P, RA[:, 0:NY], RA[:, 0:NY], F(8, 2, 2 + NY), AL.add)
                tt(P, RB[:, 0:NY], F(3, 1, 1 + NY), F(6, 0, NY), AL.add)
                tt(P, RB[:, 0:NY], RB[:, 0:NY], F(7, 2, 2 + NY), AL.add)
                tt(V, R0[:, 0:NY], F(0, 1, 1 + NY), F(2, 0, NY), AL.add)
                tt(V, R0[:, 0:NY], R0[:, 0:NY], F(4, 2, 2 + NY), AL.add)
                tt(V, MA_[:, 0:NY], F(5, 0, NY), F(8, 2, 2 + NY), AL.subtract)
                tt(P, M0[:, 0:NY], F(2, 0, NY), F(4, 2, 2 + NY), AL.subtract)
                tt(P, MB[:, 0:NY], F(6, 0, NY), F(7, 2, 2 + NY), AL.subtract)

                sm1 = S("sv", wdt=NY); sm2 = S("dv", wdt=NY)
                inv = S("inv", FP32, wdt=NY)
                for c in range(NCH):
                    cs = slice(W * c, W * (c + 1))
                    rp = psS.tile([128, W], FP32, tag="sp0", name=f"rp{c}")
                    nc.tensor.matmul(rp[:], wblk(-1), RA[:, cs], start=True, stop=False)
                    nc.tensor.matmul(rp[:], wblk(0), R0[:, cs], start=False, stop=False)
                    nc.tensor.matmul(rp[:], wblk(1), RB[:, cs], start=False, stop=True)
                    m1p = psS.tile([128, W], FP32, tag="sp1", name=f"m1p{c}")
                    nc.tensor.matmul(m1p[:], wblk(-1), RA[:, cs], start=True, stop=False)
                    nc.tensor.matmul(m1p[:], wts[:, 3 * 128:4 * 128], RB[:, cs],
                                     start=False, stop=True)
                    m2p = psS.tile([128, W], FP32, tag="sp2", name=f"m2p{c}")
                    nc.tensor.matmul(m2p[:], wblk(-1), MA_[:, cs], start=True, stop=False)
                    nc.tensor.matmul(m2p[:], wblk(0), M0[:, cs], start=False, stop=False)
                    nc.tensor.matmul(m2p[:], wblk(1), MB[:, cs], start=False, stop=True)
                    nc.scalar.copy(OUT2[:, cs], rp[:])
                    V.reciprocal_approx_fast(inv[:, cs], rp[:])
                    nc.scalar.copy(sm1[:, cs], m1p[:])
                    nc.scalar.copy(sm2[:, cs], m2p[:])
                invb = S("invb", wdt=NY)
                nc.scalar.copy(invb[:], inv[:])
                tt(V, OUT2[:, NY:2 * NY], sm1[:], invb[:], AL.mult)
                tt(V, OUT2[:, 2 * NY:3 * NY], sm2[:], invb[:], AL.mult)

                nc.sync.dma_start(
                    out=out_d[0:9, tb:tb + 128, :].rearrange("c p y -> p c y"),
                    in_=IN[:].rearrange("p (c y) -> p c y", c=12)[:, 0:9, 1:1 + NY])
                nc.sync.dma_start(
                    out=out_d[9:12, tb + 1:tb + 127, :].rearrange("c p y -> p c y"),
                    in_=OUT2[1:127, :].rearrange("p (c y) -> p c y", c=3))

            pIN = mki.tile([FXP, 12 * FXF], BF16, tag="pIN")
            pMK = mki.tile([FXP, 3 * FXF], U8, tag="pMK")
            nc.sync.dma_start(
                out=pIN[:].rearrange("p (c y) -> p c y", c=12),
                in_=pfin_d[:, :, :].rearrange("c p y -> p c y"))
            nc.sync.dma_start(
                out=pMK[:].rearrange("p (c y) -> p c y", c=3),
                in_=pmk_d[:, :, :].rearrange("c p y -> p c y"))

            def pF(i, a=0, b=FXF):
                return pIN[:, i * FXF + a:i * FXF + b]
            pUX = pF(9); pUY = pF(10); pRH = pF(11)
            def pMKV(s, a, b):
                return pMK[:, (s + 1) * FXF + a:(s + 1) * FXF + b]

            def PS(name, dt=BF16, wdt=FXF):
                return scr.tile([FXP, wdt], dt, tag=f"p_{name}", name=f"p_{name}")

            r1 = PS("r1"); r2 = PS("r2"); t1 = PS("t1"); t2 = PS("t2")
            sv = PS("sv"); dv = PS("dv"); rs = PS("rs"); rd = PS("rd")
            a5 = PS("a5"); a6 = PS("a6"); uq = PS("uq")
            V = nc.vector; P = nc.gpsimd

            tt(P, r1[:], pRH, pUX, AL.mult)
            tt(P, r2[:], pRH, pUY, AL.mult)
            tt(P, t1[:], pUX, r1[:], AL.mult)
            tt(P, t2[:], pUY, r2[:], AL.mult)
            tt(V, uq[:], t1[:], t2[:], AL.add)
            V.tensor_scalar_mul(uq[:], uq[:], -1.5)
            tt(V, uq[:], uq[:], pRH, AL.add)
            tt(P, sv[:], pUX, pUY, AL.add)
            tt(P, dv[:], pUX, pUY, AL.subtract)
            tt(V, rs[:], r1[:], r2[:], AL.add)
            tt(V, rd[:], r1[:], r2[:], AL.subtract)
            tt(P, a5[:], sv[:], rs[:], AL.mult)
            tt(P, a6[:], dv[:], rd[:], AL.mult)
            V.tensor_scalar_mul(t1[:], t1[:], 4.5 * W1P)
            V.tensor_scalar_mul(t2[:], t2[:], 4.5 * W1P)
            V.tensor_scalar_mul(a5[:], a5[:], 4.5 * W5P)
            V.tensor_scalar_mul(a6[:], a6[:], 4.5 * W5P)
            V.tensor_scalar_mul(r1[:], r1[:], 3.0 * W1P)
            V.tensor_scalar_mul(r2[:], r2[:], 3.0 * W1P)
            V.tensor_scalar_mul(rs[:], rs[:], 3.0 * W5P)
            V.tensor_scalar_mul(rd[:], rd[:], 3.0 * W5P)
            V.tensor_scalar_mul(sv[:], uq[:], W1P)
            V.tensor_scalar_mul(dv[:], uq[:], W5P)
            V.tensor_scalar_mul(uq[:], uq[:], W0P)
            tt(P, t1[:], t1[:], sv[:], AL.add)
            tt(P, t2[:], t2[:], sv[:], AL.add)
            tt(P, a5[:], a5[:], dv[:], AL.add)
            tt(P, a6[:], a6[:], dv[:], AL.add)
            tt(V, pF(0), pF(0), uq[:], AL.add)
            tt(V, pF(1), pF(1), t1[:], AL.add)
            tt(V, pF(1), pF(1), r1[:], AL.add)
            tt(V, pF(3), pF(3), t1[:], AL.add)
            tt(V, pF(3), pF(3), r1[:], AL.subtract)
            tt(V, pF(2), pF(2), t2[:], AL.add)
            tt(V, pF(2), pF(2), r2[:], AL.add)
            tt(P, pF(4), pF(4), t2[:], AL.add)
            tt(P, pF(4), pF(4), r2[:], AL.subtract)
            tt(P, pF(5), pF(5), a5[:], AL.add)
            tt(P, pF(5), pF(5), rs[:], AL.add)
            tt(P, pF(7), pF(7), a5[:], AL.add)
            tt(P, pF(7), pF(7), rs[:], AL.subtract)
            tt(P, pF(6), pF(6), a6[:], AL.add)
            tt(P, pF(6), pF(6), rd[:], AL.subtract)
            tt(P, pF(8), pF(8), a6[:], AL.add)
            tt(P, pF(8), pF(8), rd[:], AL.add)

            for i, j in ((1, 3), (2, 4), (5, 7), (6, 8)):
                bbs = {}
                for d, tag in ((i, "bb0"), (j, "bb1")):
                    exd, eyd = EX[d], EY[d]
                    pa, ma, oa = _pamaoa(eyd)
                    bb = PS(tag, wdt=FXW)
                    sp = psS.tile([FXP, FXW], FP32, tag=f"psp{0 if d == i else 1}",
                                  name=f"psp{d}")
                    nc.tensor.matmul(sp[:], pwblk(exd), pF(OPP[d], ma, ma + FXW))
                    nc.scalar.copy(bb[:], sp[:])
                    bbs[d] = bb
                for d in (i, j):
                    exd, eyd = EX[d], EY[d]
                    pa, ma, oa = _pamaoa(eyd)
                    V.copy_predicated(pF(d, oa, oa + FXW),
                                      pMKV(exd, pa, pa + FXW), bbs[d][:])

            RA = PS("r1"); RB = PS("r2"); R0 = PS("t1")
            MA_ = PS("t2"); M0 = PS("a5"); MB = PS("a6")
            tt(P, RA[:, 0:FXW], pF(1, 1, 1 + FXW), pF(5, 0, FXW), AL.add)
            tt(P, RA[:, 0:FXW], RA[:, 0:FXW], pF(8, 2, 2 + FXW), AL.add)
            tt(P, RB[:, 0:FXW], pF(3, 1, 1 + FXW), pF(6, 0, FXW), AL.add)
            tt(P, RB[:, 0:FXW], RB[:, 0:FXW], pF(7, 2, 2 + FXW), AL.add)
            tt(V, R0[:, 0:FXW], pF(0, 1, 1 + FXW), pF(2, 0, FXW), AL.add)
            tt(V, R0[:, 0:FXW], R0[:, 0:FXW], pF(4, 2, 2 + FXW), AL.add)
            tt(V, MA_[:, 0:FXW], pF(5, 0, FXW), pF(8, 2, 2 + FXW), AL.subtract)
            tt(P, M0[:, 0:FXW], pF(2, 0, FXW), pF(4, 2, 2 + FXW), AL.subtract)
            tt(P, MB[:, 0:FXW], pF(6, 0, FXW), pF(7, 2, 2 + FXW), AL.subtract)

            rp = psS.tile([FXP, FXW], FP32, tag="psp0", name="prp")
            nc.tensor.matmul(rp[:], pwblk(-1), RA[:, 0:FXW], start=True, stop=False)
            nc.tensor.matmul(rp[:], pwblk(0), R0[:, 0:FXW], start=False, stop=False)
            nc.tensor.matmul(rp[:], pwblk(1), RB[:, 0:FXW], start=False, stop=True)
            m1p = psS.tile([FXP, FXW], FP32, tag="psp1", name="pm1p")
            nc.tensor.matmul(m1p[:], pwblk(-1), RA[:, 0:FXW], start=True, stop=False)
            nc.tensor.matmul(m1p[:], pwts[:, 3 * FXP:4 * FXP], RB[:, 0:FXW],
                             start=False, stop=True)
            m2p = psS.tile([FXP, FXW], FP32, tag="psp2", name="pm2p")
            nc.tensor.matmul(m2p[:], pwblk(-1), MA_[:, 0:FXW], start=True, stop=False)
            nc.tensor.matmul(m2p[:], pwblk(0), M0[:, 0:FXW], start=False, stop=False)
            nc.tensor.matmul(m2p[:], pwblk(1), MB[:, 0:FXW], start=False, stop=True)
            pinv = PS("pinv", FP32, wdt=FXW)
            psm1 = PS("sv", wdt=FXW); psm2 = PS("dv", wdt=FXW)
            nc.scalar.copy(pF(9, 1, 1 + FXW), rp[:])
            V.reciprocal_approx_fast(pinv[:], rp[:])
            nc.scalar.copy(psm1[:], m1p[:])
            nc.scalar.copy(psm2[:], m2p[:])
            pinvb = PS("pinvb", wdt=FXW)
            nc.scalar.copy(pinvb[:], pinv[:])
            tt(V, pF(10, 1, 1 + FXW), psm1[:], pinvb[:], AL.mult)
            tt(V, pF(11, 1, 1 + FXW), psm2[:], pinvb[:], AL.mult)

            for sg in range(FXSEG):
                nc.sync.dma_start(
                    out=out_d[:, 127:131, sg * FXW:(sg + 1) * FXW].rearrange(
                        "c r y -> r c y"),
                    in_=pIN[sg * FXNR + 1:sg * FXNR + 5, :].rearrange(
                        "p (c y) -> p c y", c=12)[:, :, 1:1 + FXW])

    nc.finalize()
    return nc


_NC_CACHE = None


def _get_nc():
    global _NC_CACHE
    if _NC_CACHE is None:
        _NC_CACHE = _build_program()
    return _NC_CACHE


def _wts_np():
    import ml_dtypes
    m = np.zeros((128, 4 * 128), np.float32)
    for s in (-1, 0, 1):
        for q in range(128):
            k = q + s
            if 0 <= k < 128:
                m[k, (s + 1) * 128 + q] = 1.0
    for q in range(128):
        k = q + 1
        if 0 <= k < 128:
            m[k, 3 * 128 + q] = -1.0
    return m.astype(ml_dtypes.bfloat16)


def _pwts_np():
    import ml_dtypes
    m = np.zeros((FXP, 4 * FXP), np.float32)
    for s in (-1, 0, 1):
        for sg in range(FXSEG):
            for j in range(FXNR):
                q = sg * FXNR + j
                jk = j + s
                if 0 <= jk < FXNR:
                    m[sg * FXNR + jk, (s + 1) * FXP + q] = 1.0
    for sg in range(FXSEG):
        for j in range(FXNR):
            q = sg * FXNR + j
            jk = j + 1
            if 0 <= jk < FXNR:
                m[sg * FXNR + jk, 3 * FXP + q] = -1.0
    return m.astype(ml_dtypes.bfloat16)


def _prep_inputs(f, rho, u, obstacle_mask):
    import ml_dtypes
    f = np.asarray(f, dtype=np.float32)
    rho = np.asarray(rho, dtype=np.float32)
    u = np.asarray(u, dtype=np.float32)
    mask = np.asarray(obstacle_mask).astype(np.uint8)

    planes = np.empty((12, NX, NY), np.float32)
    for i in range(9):
        planes[i] = FCOEF * f[..., i]
    planes[9] = u[..., 0]
    planes[10] = u[..., 1]
    planes[11] = rho
    planes_b = planes.astype(ml_dtypes.bfloat16)

    wts = _wts_np()
    pwts = _pwts_np()
    rows_idx = np.arange(-1, R + 1)
    cols_idx = np.arange(-1, NY + 1) % NY
    in_maps = []
    for k in range(NCORES):
        lo = k * R
        ridx = (lo + rows_idx) % NX
        fin = planes_b[:, ridx][:, :, cols_idx]
        mk = np.empty((3, SLAB, YP), np.uint8)
        for si, s in enumerate((-1, 0, 1)):
            mk[si] = mask[(lo + rows_idx + s) % NX][:, cols_idx]
        pfin = np.empty((12, FXP, FXF), ml_dtypes.bfloat16)
        pmk = np.empty((3, FXP, FXF), np.uint8)
        frows = (lo - 1 + FXR0 + np.arange(FXNR)) % NX
        for sg in range(FXSEG):
            ccols = (sg * FXW + np.arange(-1, FXW + 1)) % NY
            seg = planes_b[:, frows][:, :, ccols]
            pfin[:, sg * FXNR:(sg + 1) * FXNR] = seg
            for si, s in enumerate((-1, 0, 1)):
                pmk[si, sg * FXNR:(sg + 1) * FXNR] = \
                    mask[(frows + s) % NX][:, ccols]
        in_maps.append({
            "fin": np.ascontiguousarray(fin),
            "mk": np.ascontiguousarray(mk),
            "wts": wts,
            "pfin": np.ascontiguousarray(pfin),
            "pmk": np.ascontiguousarray(pmk),
            "pwts": pwts,
        })
    return in_maps


def kernel(f, rho, u, obstacle_mask, _trace=False):
    in_maps = _prep_inputs(f, rho, u, obstacle_mask)
    nc = _get_nc()
    res = run_bass_kernel_spmd(nc, in_maps, list(range(NCORES)),
                               trace=bool(_trace))
    full = np.empty((NX, NY, 12), np.float32)
    chan = np.concatenate(
        [np.asarray(res.results[k]["out"])[:, 1:R + 1, :].astype(np.float32)
         for k in range(NCORES)], axis=1)
    for c in range(9):
        full[..., c] = np.roll(chan[c], (EX[c], EY[c]), axis=(0, 1))
    full[..., 9] = chan[9]
    full[..., 10] = chan[10]
    full[..., 11] = chan[11]
    if _trace:
        return full, res
    return full


# revision 9
# speedup vs baseline: 1.6709x; 1.0116x over previous
import numpy as np
import concourse.bass as bass
import concourse.bacc as bacc
import concourse.mybir as mybir
from concourse import tile
from concourse.bass_utils import run_bass_kernel_spmd

NX = 2048
NY = 2048
NCORES = 8
R = NX // NCORES
SLAB = R + 2
YP = NY + 2
TB = [0, 130]
W = 512
NCH = NY // W

TAU = 0.6
INV_TAU = 1.0 / TAU
FCOEF = 1.0 - INV_TAU
W1P = INV_TAU * (1.0 / 9.0)
W5P = INV_TAU * (1.0 / 36.0)
W0P = INV_TAU * (4.0 / 9.0)

EX = [0, 1, 0, -1, 0, 1, -1, -1, 1]
EY = [0, 0, 1, 0, -1, 1, 1, -1, -1]
OPP = [0, 3, 4, 1, 2, 7, 8, 5, 6]

FXR0 = 126
FXNR = 6
FXSEG = 8
FXW = NY // FXSEG
FXF = FXW + 2
FXP = FXSEG * FXNR

FP32 = mybir.dt.float32
BF16 = mybir.dt.bfloat16
U8 = mybir.dt.uint8
AL = mybir.AluOpType

def _pamaoa(ey):
    pa = 1 + ey
    return pa, pa, 1


def _build_program():
    nc = bacc.Bacc(None)

    fin_d = nc.declare_dram_parameter("fin", [12, SLAB, YP], BF16, isOutput=False)
    mk_d = nc.declare_dram_parameter("mk", [3, SLAB, YP], U8, isOutput=False)
    wts_d = nc.declare_dram_parameter("wts", [128, 4 * 128], BF16, isOutput=False)
    pfin_d = nc.declare_dram_parameter("pfin", [12, FXP, FXF], BF16, isOutput=False)
    pmk_d = nc.declare_dram_parameter("pmk", [3, FXP, FXF], U8, isOutput=False)
    pwts_d = nc.declare_dram_parameter("pwts", [FXP, 4 * FXP], BF16, isOutput=False)
    out_d = nc.declare_dram_parameter("out", [12, SLAB, NY], BF16, isOutput=True)

    def tt(eng, o, a, b, op):
        eng.tensor_tensor(o, a, b, op)

    with tile.TileContext(nc) as tc, tc.tile_pool(name="cst", bufs=1) as cst:
        wts = cst.tile([128, 4 * 128], BF16)
        pwts = cst.tile([FXP, 4 * FXP], BF16)
        nc.sync.dma_start(out=wts[:], in_=wts_d[:, :])
        nc.sync.dma_start(out=pwts[:], in_=pwts_d[:, :])
        def wblk(s):
            return wts[:, (s + 1) * 128:(s + 2) * 128]
        def pwblk(s):
            return pwts[:, (s + 1) * FXP:(s + 2) * FXP]

        with (
            tc.tile_pool(name="io", bufs=2) as io,
            tc.tile_pool(name="mki", bufs=1) as mki,
            tc.tile_pool(name="o2", bufs=1) as o2p,
            tc.tile_pool(name="psS", bufs=1, space="PSUM") as psS,
            tc.tile_pool(name="scr", bufs=1) as scr,
        ):
            for tb in TB:
                IN = io.tile([128, 12 * YP], BF16, tag="IN")
                MK = mki.tile([128, 3 * YP], U8, tag="MK")
                OUT2 = o2p.tile([128, 3 * NY], BF16, tag="OUT2")
                nc.sync.dma_start(
                    out=IN[:].rearrange("p (c y) -> p c y", c=12),
                    in_=fin_d[:, tb:tb + 128, :].rearrange("c p y -> p c y"))
                nc.sync.dma_start(
                    out=MK[:].rearrange("p (c y) -> p c y", c=3),
                    in_=mk_d[:, tb:tb + 128, :].rearrange("c p y -> p c y"))

                def F(i, a=0, b=YP):
                    return IN[:, i * YP + a:i * YP + b]
                UX = F(9); UY = F(10); RH = F(11)
                def MKV(s, a, b):
                    return MK[:, (s + 1) * YP + a:(s + 1) * YP + b]

                def S(name, dt=BF16, wdt=YP):
                    return scr.tile([128, wdt], dt, tag=name, name=name)

                r1 = S("r1"); r2 = S("r2"); t1 = S("t1"); t2 = S("t2")
                sv = S("sv"); dv = S("dv"); rs = S("rs"); rd = S("rd")
                a5 = S("a5"); a6 = S("a6"); uq = S("uq")
                V = nc.vector; P = nc.gpsimd

                tt(P, r1[:], RH, UX, AL.mult)
                tt(P, r2[:], RH, UY, AL.mult)
                tt(P, t1[:], UX, r1[:], AL.mult)
                tt(P, t2[:], UY, r2[:], AL.mult)
                tt(V, uq[:], t1[:], t2[:], AL.add)
                V.tensor_scalar_mul(uq[:], uq[:], -1.5)
                tt(V, uq[:], uq[:], RH, AL.add)
                tt(P, sv[:], UX, UY, AL.add)
                tt(P, dv[:], UX, UY, AL.subtract)
                tt(V, rs[:], r1[:], r2[:], AL.add)
                tt(V, rd[:], r1[:], r2[:], AL.subtract)
                tt(P, a5[:], sv[:], rs[:], AL.mult)
                tt(P, a6[:], dv[:], rd[:], AL.mult)
                V.tensor_scalar_mul(t1[:], t1[:], 4.5 * W1P)
                V.tensor_scalar_mul(t2[:], t2[:], 4.5 * W1P)
                V.tensor_scalar_mul(a5[:], a5[:], 4.5 * W5P)
                V.tensor_scalar_mul(a6[:], a6[:], 4.5 * W5P)
                V.tensor_scalar_mul(r1[:], r1[:], 3.0 * W1P)
                V.tensor_scalar_mul(r2[:], r2[:], 3.0 * W1P)
                V.tensor_scalar_mul(rs[:], rs[:], 3.0 * W5P)
                V.tensor_scalar_mul(rd[:], rd[:], 3.0 * W5P)
                V.tensor_scalar_mul(sv[:], uq[:], W1P)
                V.tensor_scalar_mul(dv[:], uq[:], W5P)
                V.tensor_scalar_mul(uq[:], uq[:], W0P)
                tt(P, t1[:], t1[:], sv[:], AL.add)
                tt(P, t2[:], t2[:], sv[:], AL.add)
                tt(P, a5[:], a5[:], dv[:], AL.add)
                tt(P, a6[:], a6[:], dv[:], AL.add)
                tt(V, F(0), F(0), uq[:], AL.add)
                tt(V, F(1), F(1), t1[:], AL.add)
                tt(V, F(1), F(1), r1[:], AL.add)
                tt(V, F(3), F(3), t1[:], AL.add)
                tt(V, F(3), F(3), r1[:], AL.subtract)
                tt(V, F(2), F(2), t2[:], AL.add)
                tt(V, F(2), F(2), r2[:], AL.add)
                tt(P, F(4), F(4), t2[:], AL.add)
                tt(P, F(4), F(4), r2[:], AL.subtract)
                tt(P, F(5), F(5), a5[:], AL.add)
                tt(P, F(5), F(5), rs[:], AL.add)
                tt(P, F(7), F(7), a5[:], AL.add)
                tt(P, F(7), F(7), rs[:], AL.subtract)
                tt(P, F(6), F(6), a6[:], AL.add)
                tt(P, F(6), F(6), rd[:], AL.subtract)
                tt(P, F(8), F(8), a6[:], AL.add)
                tt(P, F(8), F(8), rd[:], AL.add)

                for i, j in ((1, 3), (2, 4), (5, 7), (6, 8)):
                    bbs = {}
                    for d, tag in ((i, "bb0"), (j, "bb1")):
                        exd, eyd = EX[d], EY[d]
                        pa, ma, oa = _pamaoa(eyd)
                        bb = S(tag, wdt=NY)
                        for c in range(NCH):
                            sp = psS.tile([128, W], FP32, tag=f"sp{c % 4}",
                                          name=f"sp{d}_{c}")
                            nc.tensor.matmul(sp[:], wblk(exd),
                                             F(OPP[d], ma + W * c, ma + W * (c + 1)))
                            nc.scalar.copy(bb[:, W * c:W * (c + 1)], sp[:])
                        bbs[d] = bb
                    for d in (i, j):
                        exd, eyd = EX[d], EY[d]
                        pa, ma, oa = _pamaoa(eyd)
                        V.copy_predicated(F(d, oa, oa + NY),
                                          MKV(exd, pa, pa + NY), bbs[d][:])

                for i in (2, 5, 6):
                    nc.scalar.copy(F(i, 0, 1), F(i, NY, NY + 1))
                for i in (4, 7, 8):
                    nc.scalar.copy(F(i, NY + 1, NY + 2), F(i, 1, 2))

                RA = S("r1"); RB = S("r2"); R0 = S("t1")
                MA_ = S("t2"); M0 = S("a5"); MB = S("a6")
                tt(P, RA[:, 0:NY], F(1, 1, 1 + NY), F(5, 0, NY), AL.add)
                tt(P, RA[:, 0:NY], RA[:, 0:NY], F(8, 2, 2 + NY), AL.add)
                tt(P, RB[:, 0:NY], F(3, 1, 1 + NY), F(6, 0, NY), AL.add)
                tt(P, RB[:, 0:NY], RB[:, 0:NY], F(7, 2, 2 + NY), AL.add)
                tt(V, R0[:, 0:NY], F(0, 1, 1 + NY), F(2, 0, NY), AL.add)
                tt(V, R0[:, 0:NY], R0[:, 0:NY], F(4, 2, 2 + NY), AL.add)
                tt(V, MA_[:, 0:NY], F(5, 0, NY), F(8, 2, 2 + NY), AL.subtract)
                tt(P, M0[:, 0:NY], F(2, 0, NY), F(4, 2, 2 + NY), AL.subtract)
                tt(P, MB[:, 0:NY], F(6, 0, NY), F(7, 2, 2 + NY), AL.subtract)

                sm1 = S("sv", wdt=NY); sm2 = S("dv", wdt=NY)
                inv = S("inv", FP32, wdt=NY)
                for c in range(NCH):
                    cs = slice(W * c, W * (c + 1))
                    rp = psS.tile([128, W], FP32, tag="sp0", name=f"rp{c}")
                    nc.tensor.matmul(rp[:], wblk(-1), RA[:, cs], start=True, stop=False)
                    nc.tensor.matmul(rp[:], wblk(0), R0[:, cs], start=False, stop=False)
                    nc.tensor.matmul(rp[:], wblk(1), RB[:, cs], start=False, stop=True)
                    m1p = psS.tile([128, W], FP32, tag="sp1", name=f"m1p{c}")
                    nc.tensor.matmul(m1p[:], wblk(-1), RA[:, cs], start=True, stop=False)
                    nc.tensor.matmul(m1p[:], wts[:, 3 * 128:4 * 128], RB[:, cs],
                                     start=False, stop=True)
                    m2p = psS.tile([128, W], FP32, tag="sp2", name=f"m2p{c}")
                    nc.tensor.matmul(m2p[:], wblk(-1), MA_[:, cs], start=True, stop=False)
                    nc.tensor.matmul(m2p[:], wblk(0), M0[:, cs], start=False, stop=False)
                    nc.tensor.matmul(m2p[:], wblk(1), MB[:, cs], start=False, stop=True)
                    nc.scalar.copy(OUT2[:, cs], rp[:])
                    V.reciprocal_approx_fast(inv[:, cs], rp[:])
                    nc.scalar.copy(sm1[:, cs], m1p[:])
                    nc.scalar.copy(sm2[:, cs], m2p[:])
                invb = S("invb", wdt=NY)
                nc.scalar.copy(invb[:], inv[:])
                tt(V, OUT2[:, NY:2 * NY], sm1[:], invb[:], AL.mult)
                tt(V, OUT2[:, 2 * NY:3 * NY], sm2[:], invb[:], AL.mult)

                nc.sync.dma_start(
                    out=out_d[0:9, tb:tb + 128, :].rearrange("c p y -> p c y"),
                    in_=IN[:].rearrange("p (c y) -> p c y", c=12)[:, 0:9, 1:1 + NY])
                nc.sync.dma_start(
                    out=out_d[9:12, tb + 1:tb + 127, :].rearrange("c p y -> p c y"),
                    in_=OUT2[1:127, :].rearrange("p (c y) -> p c y", c=3))

            pIN = mki.tile([FXP, 12 * FXF], BF16, tag="pIN")
            pMK = mki.tile([FXP, 3 * FXF], U8, tag="pMK")
            nc.sync.dma_start(
                out=pIN[:].rearrange("p (c y) -> p c y", c=12),
                in_=pfin_d[:, :, :].rearrange("c p y -> p c y"))
            nc.sync.dma_start(
                out=pMK[:].rearrange("p (c y) -> p c y", c=3),
                in_=pmk_d[:, :, :].rearrange("c p y -> p c y"))

            def pF(i, a=0, b=FXF):
                return pIN[:, i * FXF + a:i * FXF + b]
            pUX = pF(9); pUY = pF(10); pRH = pF(11)
            def pMKV(s, a, b):
                return pMK[:, (s + 1) * FXF + a:(s + 1) * FXF + b]

            def PS(name, dt=BF16, wdt=FXF):
                return scr.tile([FXP, wdt], dt, tag=f"p_{name}", name=f"p_{name}")

            r1 = PS("r1"); r2 = PS("r2"); t1 = PS("t1"); t2 = PS("t2")
            sv = PS("sv"); dv = PS("dv"); rs = PS("rs"); rd = PS("rd")
            a5 = PS("a5"); a6 = PS("a6"); uq = PS("uq")
            V = nc.vector; P = nc.gpsimd

            tt(P, r1[:], pRH, pUX, AL.mult)
            tt(P, r2[:], pRH, pUY, AL.mult)
            tt(P, t1[:], pUX, r1[:], AL.mult)
            tt(P, t2[:], pUY, r2[:], AL.mult)
            tt(V, uq[:], t1[:], t2[:], AL.add)
            V.tensor_scalar_mul(uq[:], uq[:], -1.5)
            tt(V, uq[:], uq[:], pRH, AL.add)
            tt(P, sv[:], pUX, pUY, AL.add)
            tt(P, dv[:], pUX, pUY, AL.subtract)
            tt(V, rs[:], r1[:], r2[:], AL.add)
            tt(V, rd[:], r1[:], r2[:], AL.subtract)
            tt(P, a5[:], sv[:], rs[:], AL.mult)
            tt(P, a6[:], dv[:], rd[:], AL.mult)
            V.tensor_scalar_mul(t1[:], t1[:], 4.5 * W1P)
            V.tensor_scalar_mul(t2[:], t2[:], 4.5 * W1P)
            V.tensor_scalar_mul(a5[:], a5[:], 4.5 * W5P)
            V.tensor_scalar_mul(a6[:], a6[:], 4.5 * W5P)
            V.tensor_scalar_mul(r1[:], r1[:], 3.0 * W1P)
            V.tensor_scalar_mul(r2[:], r2[:], 3.0 * W1P)
            V.tensor_scalar_mul(rs[:], rs[:], 3.0 * W5P)
            V.tensor_scalar_mul(rd[:], rd[:], 3.0 * W5P)
            V.tensor_scalar_mul(sv[:], uq[:], W1P)
            V.tensor_scalar_mul(dv[:], uq[:], W5P)
            V.tensor_scalar_mul(uq[:], uq[:], W0P)
            tt(P, t1[:], t1[:], sv[:], AL.add)
            tt(P, t2[:], t2[:], sv[:], AL.add)
            tt(P, a5[:], a5[:], dv[:], AL.add)
            tt(P, a6[:], a6[:], dv[:], AL.add)
            tt(V, pF(0), pF(0), uq[:], AL.add)
            tt(V, pF(1), pF(1), t1[:], AL.add)
            tt(V, pF(1), pF(1), r1[:], AL.add)
            tt(V, pF(3), pF(3), t1[:], AL.add)
            tt(V, pF(3), pF(3), r1[:], AL.subtract)
            tt(V, pF(2), pF(2), t2[:], AL.add)
            tt(V, pF(2), pF(2), r2[:], AL.add)
            tt(P, pF(4), pF(4), t2[:], AL.add)
            tt(P, pF(4), pF(4), r2[:], AL.subtract)
            tt(P, pF(5), pF(5), a5[:], AL.add)
            tt(P, pF(5), pF(5), rs[:], AL.add)
            tt(P, pF(7), pF(7), a5[:], AL.add)
            tt(P, pF(7), pF(7), rs[:], AL.subtract)
            tt(P, pF(6), pF(6), a6[:], AL.add)
            tt(P, pF(6), pF(6), rd[:], AL.subtract)
            tt(P, pF(8), pF(8), a6[:], AL.add)
            tt(P, pF(8), pF(8), rd[:], AL.add)

            for i, j in ((1, 3), (2, 4), (5, 7), (6, 8)):
                bbs = {}
                for d, tag in ((i, "bb0"), (j, "bb1")):
                    exd, eyd = EX[d], EY[d]
                    pa, ma, oa = _pamaoa(eyd)
                    bb = PS(tag, wdt=FXW)
                    sp = psS.tile([FXP, FXW], FP32, tag=f"psp{0 if d == i else 1}",
                                  name=f"psp{d}")
                    nc.tensor.matmul(sp[:], pwblk(exd), pF(OPP[d], ma, ma + FXW))
                    nc.scalar.copy(bb[:], sp[:])
                    bbs[d] = bb
                for d in (i, j):
                    exd, eyd = EX[d], EY[d]
                    pa, ma, oa = _pamaoa(eyd)
                    V.copy_predicated(pF(d, oa, oa + FXW),
                                      pMKV(exd, pa, pa + FXW), bbs[d][:])

            RA = PS("r1"); RB = PS("r2"); R0 = PS("t1")
            MA_ = PS("t2"); M0 = PS("a5"); MB = PS("a6")
            tt(P, RA[:, 0:FXW], pF(1, 1, 1 + FXW), pF(5, 0, FXW), AL.add)
            tt(P, RA[:, 0:FXW], RA[:, 0:FXW], pF(8, 2, 2 + FXW), AL.add)
            tt(P, RB[:, 0:FXW], pF(3, 1, 1 + FXW), pF(6, 0, FXW), AL.add)
            tt(P, RB[:, 0:FXW], RB[:, 0:FXW], pF(7, 2, 2 + FXW), AL.add)
            tt(V, R0[:, 0:FXW], pF(0, 1, 1 + FXW), pF(2, 0, FXW), AL.add)
            tt(V, R0[:, 0:FXW], R0[:, 0:FXW], pF(4, 2, 2 + FXW), AL.add)
            tt(V, MA_[:, 0:FXW], pF(5, 0, FXW), pF(8, 2, 2 + FXW), AL.subtract)
            tt(P, M0[:, 0:FXW], pF(2, 0, FXW), pF(4, 2, 2 + FXW), AL.subtract)
            tt(P, MB[:, 0:FXW], pF(6, 0, FXW), pF(7, 2, 2 + FXW), AL.subtract)

            rp = psS.tile([FXP, FXW], FP32, tag="psp0", name="prp")
            nc.tensor.matmul(rp[:], pwblk(-1), RA[:, 0:FXW], start=True, stop=False)
            nc.tensor.matmul(rp[:], pwblk(0), R0[:, 0:FXW], start=False, stop=False)
            nc.tensor.matmul(rp[:], pwblk(1), RB[:, 0:FXW], start=False, stop=True)
            m1p = psS.tile([FXP, FXW], FP32, tag="psp1", name="pm1p")
            nc.tensor.matmul(m1p[:], pwblk(-1), RA[:, 0:FXW], start=True, stop=False)
            nc.tensor.matmul(m1p[:], pwts[:, 3 * FXP:4 * FXP], RB[:, 0:FXW],
                             start=False, stop=True)
            m2p = psS.tile([FXP, FXW], FP32, tag="psp2", name="pm2p")
            nc.tensor.matmul(m2p[:], pwblk(-1), MA_[:, 0:FXW], start=True, stop=False)
            nc.tensor.matmul(m2p[:], pwblk(0), M0[:, 0:FXW], start=False, stop=False)
            nc.tensor.matmul(m2p[:], pwblk(1), MB[:, 0:FXW], start=False, stop=True)
            pinv = PS("pinv", FP32, wdt=FXW)
            psm1 = PS("sv", wdt=FXW); psm2 = PS("dv", wdt=FXW)
            nc.scalar.copy(pF(9, 1, 1 + FXW), rp[:])
            V.reciprocal_approx_fast(pinv[:], rp[:])
            nc.scalar.copy(psm1[:], m1p[:])
            nc.scalar.copy(psm2[:], m2p[:])
            pinvb = PS("pinvb", wdt=FXW)
            nc.scalar.copy(pinvb[:], pinv[:])
            tt(V, pF(10, 1, 1 + FXW), psm1[:], pinvb[:], AL.mult)
            tt(V, pF(11, 1, 1 + FXW), psm2[:], pinvb[:], AL.mult)

            for sg in range(FXSEG):
                nc.sync.dma_start(
                    out=out_d[:, 127:131, sg * FXW:(sg + 1) * FXW].rearrange(
                        "c r y -> r c y"),
                    in_=pIN[sg * FXNR + 1:sg * FXNR + 5, :].rearrange(
                        "p (c y) -> p c y", c=12)[:, :, 1:1 + FXW])

    nc.finalize()
    return nc


_NC_CACHE = None


def _get_nc():
    global _NC_CACHE
    if _NC_CACHE is None:
        _NC_CACHE = _build_program()
    return _NC_CACHE


def _wts_np():
    import ml_dtypes
    m = np.zeros((128, 4 * 128), np.float32)
    for s in (-1, 0, 1):
        for q in range(128):
            k = q + s
            if 0 <= k < 128:
                m[k, (s + 1) * 128 + q] = 1.0
    for q in range(128):
        k = q + 1
        if 0 <= k < 128:
            m[k, 3 * 128 + q] = -1.0
    return m.astype(ml_dtypes.bfloat16)


def _pwts_np():
    import ml_dtypes
    m = np.zeros((FXP, 4 * FXP), np.float32)
    for s in (-1, 0, 1):
        for sg in range(FXSEG):
            for j in range(FXNR):
                q = sg * FXNR + j
                jk = j + s
                if 0 <= jk < FXNR:
                    m[sg * FXNR + jk, (s + 1) * FXP + q] = 1.0
    for sg in range(FXSEG):
        for j in range(FXNR):
            q = sg * FXNR + j
            jk = j + 1
            if 0 <= jk < FXNR:
                m[sg * FXNR + jk, 3 * FXP + q] = -1.0
    return m.astype(ml_dtypes.bfloat16)


def _prep_inputs(f, rho, u, obstacle_mask):
    import ml_dtypes
    f = np.asarray(f, dtype=np.float32)
    rho = np.asarray(rho, dtype=np.float32)
    u = np.asarray(u, dtype=np.float32)
    mask = np.asarray(obstacle_mask).astype(np.uint8)

    planes = np.empty((12, NX, NY), np.float32)
    for i in range(9):
        planes[i] = FCOEF * f[..., i]
    planes[9] = u[..., 0]
    planes[10] = u[..., 1]
    planes[11] = rho
    planes_b = planes.astype(ml_dtypes.bfloat16)

    wts = _wts_np()
    pwts = _pwts_np()
    rows_idx = np.arange(-1, R + 1)
    cols_idx = np.arange(-1, NY + 1) % NY
    in_maps = []
    for k in range(NCORES):
        lo = k * R
        ridx = (lo + rows_idx) % NX
        fin = planes_b[:, ridx][:, :, cols_idx]
        mk = np.empty((3, SLAB, YP), np.uint8)
        for si, s in enumerate((-1, 0, 1)):
            mk[si] = mask[(lo + rows_idx + s) % NX][:, cols_idx]
        pfin = np.empty((12, FXP, FXF), ml_dtypes.bfloat16)
        pmk = np.empty((3, FXP, FXF), np.uint8)
        frows = (lo - 1 + FXR0 + np.arange(FXNR)) % NX
        for sg in range(FXSEG):
            ccols = (sg * FXW + np.arange(-1, FXW + 1)) % NY
            seg = planes_b[:, frows][:, :, ccols]
            pfin[:, sg * FXNR:(sg + 1) * FXNR] = seg
            for si, s in enumerate((-1, 0, 1)):
                pmk[si, sg * FXNR:(sg + 1) * FXNR] = \
                    mask[(frows + s) % NX][:, ccols]
        in_maps.append({
            "fin": np.ascontiguousarray(fin),
            "mk": np.ascontiguousarray(mk),
            "wts": wts,
            "pfin": np.ascontiguousarray(pfin),
            "pmk": np.ascontiguousarray(pmk),
            "pwts": pwts,
        })
    return in_maps


def kernel(f, rho, u, obstacle_mask, _trace=False):
    in_maps = _prep_inputs(f, rho, u, obstacle_mask)
    nc = _get_nc()
    res = run_bass_kernel_spmd(nc, in_maps, list(range(NCORES)),
                               trace=bool(_trace))
    full = np.empty((NX, NY, 12), np.float32)
    chan = np.concatenate(
        [np.asarray(res.results[k]["out"])[:, 1:R + 1, :].astype(np.float32)
         for k in range(NCORES)], axis=1)
    for c in range(9):
        full[..., c] = np.roll(chan[c], (EX[c], EY[c]), axis=(0, 1))
    full[..., 9] = chan[9]
    full[..., 10] = chan[10]
    full[..., 11] = chan[11]
    if _trace:
        return full, res
    return full
